# revision 12
# baseline (speedup 1.0000x reference)
"""ClassAttention Trainium2 kernel (8-core data-parallel over batch).

Math (per sample):
  q = (x[0] @ q_w + q_b) * scale            # [C] -> heads [H, HD]
  scores[h, n] = q[h] . (x[n] @ k_w + k_b)[h-block]
              = x[n] @ QK[:, h] + const(h)   (const cancels in softmax)
    where QK[:, h] = k_w[:, hblk] @ q[h]     # folded: [C, H] instead of k [N, C]
  attn = softmax(scores, axis=n)             # output 1
  ax[h] = attn[h] @ x                        # [C]
  o[hblk] = ax[h] @ v_w[:, hblk] + v_b[hblk] # (sum attn = 1)
  x_cls = o @ proj_w + proj_b                # output 2

This avoids materializing k and v entirely (the reference's dominant cost).
"""

import numpy as np

import concourse.bass as bass
from concourse import bacc
import concourse.mybir as mybir
import concourse.tile as tile
from concourse.bass import ts
from concourse.masks import make_identity

B, N, C, H, HD = 32, 3137, 768, 12, 64
NCORES = 8
BPC = B // NCORES          # batches per core
SCALE = float(HD) ** -0.5
P = 128
CCH = C // P               # 6 contraction chunks
NT = (N + P - 1) // P      # 25 n-tiles
NTAIL = N - (NT - 1) * P   # 65
SCH = 512                  # n super-chunk (moving free dim for scores)
NSC = (N + SCH - 1) // SCH # 7
XCH = 5                    # n-tiles per x DMA chunk
NXCH = NT // XCH           # 5 chunks per batch
SCPAD = NT * P             # 3200 padded scores row

F32 = mybir.dt.float32
F32R = mybir.dt.float32r
AX = mybir.AxisListType
AF = mybir.ActivationFunctionType


def nrows(t):
    return NTAIL if t == NT - 1 else P


def build_kernel(nc: bass.Bass):
    xs = nc.dram_tensor("xs", [BPC, N, C], F32, kind="ExternalInput").ap()
    qw_d = nc.dram_tensor("q_w", [C, C], F32, kind="ExternalInput").ap()
    kw_d = nc.dram_tensor("k_w", [C, C], F32, kind="ExternalInput").ap()
    vw_d = nc.dram_tensor("v_w", [C, C], F32, kind="ExternalInput").ap()
    pw_d = nc.dram_tensor("proj_w", [C, C], F32, kind="ExternalInput").ap()
    qb_d = nc.dram_tensor("q_b", [C], F32, kind="ExternalInput").ap()
    vb_d = nc.dram_tensor("v_b", [C], F32, kind="ExternalInput").ap()
    pb_d = nc.dram_tensor("proj_b", [C], F32, kind="ExternalInput").ap()
    attn_d = nc.dram_tensor("attn", [BPC, H, N], F32, kind="ExternalOutput").ap()
    xcls_d = nc.dram_tensor("x_cls", [BPC, C], F32, kind="ExternalOutput").ap()

    with tile.TileContext(nc) as tc:
        with (
            tc.tile_pool(name="pw", bufs=2) as pw,     # q_w/k_w/k_wT/xT share slots
            tc.tile_pool(name="pwv", bufs=2) as pwv,   # v_w + proj_w resident
            tc.tile_pool(name="px", bufs=6) as px,     # x chunks [128, 5, 768]
            tc.tile_pool(name="psc", bufs=2) as psc,   # scores rows [12, 3200]
            tc.tile_pool(name="pat", bufs=2) as pat,   # attn^T [128, 25, 12]
            tc.tile_pool(name="psm", bufs=1) as psm,   # persistent smalls
            tc.tile_pool(name="psm2", bufs=2) as psm2, # per-batch smalls
            tc.tile_pool(name="ppx", bufs=2, space="PSUM") as ppx,   # 2 banks each
            tc.tile_pool(name="pps", bufs=2, space="PSUM") as pps,   # 1 bank each
            tc.tile_pool(name="ppax", bufs=1, space="PSUM") as ppax, # 2 banks
        ):
            def cpeng(i):
                return nc.vector.tensor_copy if i % 2 == 0 else nc.scalar.copy

            ident = psm.tile([P, P], F32, name="ident")
            make_identity(nc, ident)

            # broadcast biases to BPC partitions
            def bcast(ap_):
                return bass.AP(
                    tensor=ap_.tensor, offset=ap_.offset, ap=[[0, BPC]] + list(ap_.ap)
                )

            qb_sb = psm2.tile([BPC, C], F32, tag="sm", name="qb_sb")
            nc.gpsimd.dma_start(out=qb_sb, in_=bcast(qb_d))
            vb_sb = psm.tile([BPC, C], F32, name="vb_sb")
            nc.gpsimd.dma_start(out=vb_sb, in_=bcast(vb_d))
            pb_sb = psm.tile([BPC, C], F32, name="pb_sb")
            nc.gpsimd.dma_start(out=pb_sb, in_=bcast(pb_d))

            # v_w / proj_w: load early, dedicated slots
            vw_sb = pwv.tile([P, CCH, C], F32, tag="wv", name="vw_sb")
            nc.sync.dma_start(out=vw_sb, in_=vw_d.rearrange("(k p) o -> p k o", p=P))
            pjw_sb = pwv.tile([P, CCH, C], F32, tag="wv", name="pjw_sb")
            nc.sync.dma_start(out=pjw_sb, in_=pw_d.rearrange("(k p) o -> p k o", p=P))

            # x0 rows -> transpose on PE -> x0T [128 cpart, 6, BPC]
            x0n = psm2.tile([BPC, C], F32, tag="sm", name="x0n")
            nc.sync.dma_start(out=x0n, in_=xs[:, 0, :])
            x0Tp = pps.tile([P, CCH, BPC], F32, tag="s", name="x0Tp")
            for k in range(CCH):
                nc.tensor.transpose(
                    out=x0Tp[:, k, :], in_=x0n[:, ts(k, P)],
                    identity=ident[:BPC, :BPC],
                )
            x0T = psm.tile([P, CCH, BPC], F32, name="x0T")
            nc.scalar.copy(out=x0T, in_=x0Tp)

            # ---- P0: q = x0 @ q_w + q_b ----
            qw_sb = pw.tile([P, CCH, C], F32, tag="w", name="qw_sb")
            nc.sync.dma_start(out=qw_sb, in_=qw_d.rearrange("(k p) o -> p k o", p=P))
            q_sb = psm2.tile([BPC, C], F32, tag="sm", name="q_sb")
            for j in range(2):
                qp = pps.tile([BPC, 384], F32, tag="s", name="qp")
                for k in range(CCH):
                    nc.tensor.matmul(
                        qp,
                        lhsT=x0T[:, k, :],
                        rhs=qw_sb[:, k, ts(j, 384)],
                        start=(k == 0),
                        stop=(k == CCH - 1),
                    )
                nc.scalar.copy(out=q_sb[:, ts(j, 384)], in_=qp)
            nc.vector.tensor_add(out=q_sb, in0=q_sb, in1=qb_sb)

            # q^T [128, 6, BPC]
            qTp = pps.tile([P, CCH, BPC], F32, tag="s", name="qTp")
            for k in range(CCH):
                nc.tensor.transpose(
                    out=qTp[:, k, :], in_=q_sb[:, ts(k, P)], identity=ident[:BPC, :BPC]
                )
            qT_sb = psm.tile([P, CCH, BPC], F32, name="qT_sb")
            nc.scalar.copy(out=qT_sb, in_=qTp)

            # Qblk [128 co-part, 6 u, 48 (b*12+h)]: block-diagonal embedding of q
            # head h occupies co rows [64h, 64h+64): chunk u=h//2, partition half h%2.
            qblk = psm.tile([P, CCH, BPC * H], F32, name="qblk")
            nc.vector.memset(qblk, 0.0)
            qb_ap = qblk[:]
            qt_ap = qT_sb[:]
            for half in range(2):  # even heads h=2u (half 0), odd h=2u+1 (half 1)
                dst = bass.AP(
                    tensor=qb_ap.tensor,
                    offset=qb_ap.offset + half * 64 * (CCH * BPC * H) + half,
                    ap=[[CCH * BPC * H, 64], [BPC * H + 2, CCH], [H, BPC]],
                )
                src = bass.AP(
                    tensor=qt_ap.tensor,
                    offset=qt_ap.offset + half * 64 * (CCH * BPC),
                    ap=[[CCH * BPC, 64], [BPC, CCH], [1, BPC]],
                )
                nc.vector.tensor_copy(out=dst, in_=src)

            # ---- k_w^T via PE transposes ----
            kw_sb = pw.tile([P, CCH, C], F32, tag="w", name="kw_sb")
            nc.sync.dma_start(out=kw_sb, in_=kw_d.rearrange("(k p) o -> p k o", p=P))
            kwT_sb = pw.tile([P, CCH, C], F32, tag="w", name="kwT_sb")
            for u in range(CCH):  # co chunk
                tp = ppx.tile([P, CCH, P], F32, tag="xt", name="ktp")
                for v in range(CCH):  # ci chunk
                    nc.tensor.transpose(
                        out=tp[:, v, :], in_=kw_sb[:, v, ts(u, P)], identity=ident
                    )
                cpeng(u)(
                    out=kwT_sb[:, u, :].rearrange("p (v q) -> p v q", q=P), in_=tp[:]
                )

            # ---- QK fold: QK[ci, (b,h)] = sum_co k_wT[co, ci] * qblk[co, (b,h)] ----
            QK_sb = psm.tile([P, CCH, BPC * H], F32R, name="QK_sb")
            for ci in range(CCH):
                qkp = pps.tile([P, BPC * H], F32, tag="s", name="qkp")
                for u in range(CCH):
                    nc.tensor.matmul(
                        qkp,
                        lhsT=kwT_sb[:, u, ts(ci, P)],
                        rhs=qblk[:, u, :],
                        start=(u == 0),
                        stop=(u == CCH - 1),
                    )
                nc.scalar.mul(out=QK_sb[:, ci, :], in_=qkp, mul=SCALE)

            # axT accumulator [128 ci-part, 6, 48 (h*4+b)]
            axT_sb = psm.tile([P, CCH, H * BPC], F32, name="axT_sb")

            # ---- main per-batch loop ----
            for b in range(BPC):
                xch = [
                    px.tile([P, XCH, C], F32R, tag="x", name=f"x{b}_{c}")
                    for c in range(NXCH)
                ]
                for c in range(NXCH):
                    n0 = c * XCH * P
                    if c < NXCH - 1:
                        nc.sync.dma_start(
                            out=xch[c],
                            in_=xs[b, n0 : n0 + XCH * P, :].rearrange(
                                "(t p) c -> p t c", p=P
                            ).bitcast(F32R),
                        )
                    else:
                        nc.sync.dma_start(
                            out=xch[c][:, : XCH - 1, :],
                            in_=xs[b, n0 : n0 + (XCH - 1) * P, :].rearrange(
                                "(t p) c -> p t c", p=P
                            ).bitcast(F32R),
                        )
                        nc.sync.dma_start(
                            out=xch[c][:NTAIL, XCH - 1, :],
                            in_=xs[b, (NT - 1) * P :, :].bitcast(F32R),
                        )

                def xtile(t, j0=0, j1=C):
                    return xch[t // XCH][: nrows(t), t % XCH, j0:j1]

                sc = psc.tile([H, SCPAD], F32, tag="sc", name=f"sc{b}")

                # scores per super-chunk of 512 n
                for s in range(NSC):
                    tlist = list(range(4 * s, min(4 * s + 4, NT)))
                    ln = min(SCH, N - s * SCH)
                    xT = pw.tile([P, CCH, SCH], F32R, tag="w", name=f"xT{b}_{s}")
                    for i, t in enumerate(tlist):
                        r = nrows(t)
                        tp = ppx.tile([P, CCH, P], F32, tag="xt", name="xtp")
                        ceng = cpeng(s)
                        for v in range(CCH):
                            nc.tensor.transpose(
                                out=tp[:, v, :r],
                                in_=xch[t // XCH][:r, t % XCH, ts(v, P)].bitcast(F32),
                                identity=ident[:r, :r],
                            )
                        ceng(
                            out=xT[:, :, i * P : i * P + r], in_=tp[:, :, :r]
                        )
                    # fp32r needs even free counts; run the odd tail in fp32
                    sp = pps.tile([H, SCH], F32, tag="s", name="sp")
                    for v in range(CCH):
                        if ln % 2 == 0:
                            nc.tensor.matmul(
                                sp[:, :ln],
                                lhsT=QK_sb[:, v, b * H : (b + 1) * H],
                                rhs=xT[:, v, :ln],
                                start=(v == 0),
                                stop=(v == CCH - 1),
                            )
                        else:
                            nc.tensor.matmul(
                                sp[:, :ln],
                                lhsT=QK_sb[:, v, b * H : (b + 1) * H].bitcast(F32),
                                rhs=xT[:, v, :ln].bitcast(F32),
                                start=(v == 0),
                                stop=(v == CCH - 1),
                            )
                    nc.scalar.copy(out=sc[:, s * SCH : s * SCH + ln], in_=sp[:, :ln])

                # softmax along n (rows = heads)
                nmx = psm2.tile([H, 1], F32, tag="nmx", name="nmx")
                nc.vector.reduce_max(out=nmx, in_=sc[:, :N], axis=AX.X, negate=True)
                zs = psm2.tile([H, 1], F32, tag="zs", name="zs")
                nc.scalar.activation(
                    out=sc[:, :N], in_=sc[:, :N], func=AF.Exp, bias=nmx, scale=1.0,
                    accum_out=zs,
                )
                iz = psm2.tile([H, 1], F32, tag="iz", name="iz")
                nc.vector.reciprocal(out=iz, in_=zs)
                nc.vector.tensor_scalar_mul(out=sc[:, :N], in0=sc[:, :N], scalar1=iz)
                nc.sync.dma_start(out=attn_d[b], in_=sc[:, :N])

                # attn^T tiles [128 n-part, 25, 12]
                aT = pat.tile([P, NT, H], F32R, tag="at", name=f"aT{b}")
                for g in range((NT + 3) // 4):
                    tlist = list(range(4 * g, min(4 * g + 4, NT)))
                    ap_ = pps.tile([P, 4, H], F32, tag="s", name="atp")
                    for i, t in enumerate(tlist):
                        r = nrows(t)
                        nc.tensor.transpose(
                            out=ap_[:r, i, :],
                            in_=sc[:, t * P : t * P + r],
                            identity=ident[:H, :H],
                        )
                    rlast = nrows(tlist[-1])
                    if rlast == P:
                        nc.vector.tensor_copy(
                            out=aT[:, 4 * g : 4 * g + len(tlist), :],
                            in_=ap_[:, : len(tlist), :],
                        )
                    else:
                        if len(tlist) > 1:
                            nc.vector.tensor_copy(
                                out=aT[:, 4 * g : 4 * g + len(tlist) - 1, :],
                                in_=ap_[:, : len(tlist) - 1, :],
                            )
                        nc.vector.tensor_copy(
                            out=aT[:rlast, 4 * g + len(tlist) - 1, :],
                            in_=ap_[:rlast, len(tlist) - 1, :],
                        )

                # ax = attn @ x : accumulate [12, 768] over n tiles
                axp = ppax.tile([H, 2, SCH], F32, tag="ax", name="axp")
                for t in range(NT):
                    r = nrows(t)
                    for j in range(2):
                        nc.tensor.matmul(
                            axp[:, j, :384],
                            lhsT=aT[:r, t, :],
                            rhs=xtile(t, j * 384, (j + 1) * 384),
                            start=(t == 0),
                            stop=(t == NT - 1),
                            skip_group_check=True,
                        )
                ax_sb = psm2.tile([H, C], F32, tag="sm", name="ax_sb")
                nc.scalar.copy(
                    out=ax_sb.rearrange("h (j o) -> h j o", j=2), in_=axp[:, :, :384]
                )

                # ax^T into axT_sb columns h*4+b
                atp2 = pps.tile([P, CCH, H], F32, tag="s", name="atp2")
                for u in range(CCH):
                    nc.tensor.transpose(
                        out=atp2[:, u, :], in_=ax_sb[:, ts(u, P)], identity=ident[:H, :H]
                    )
                nc.vector.tensor_copy(
                    out=axT_sb[:].rearrange("p u (h b2) -> p u h b2", b2=BPC)[:, :, :, b],
                    in_=atp2[:],
                )

            # ---- final: o = blockdiag(ax @ v_w) + v_b ; x_cls = o @ proj_w + proj_b
            o_sb = psm2.tile([BPC, C], F32, tag="sm", name="o_sb")
            for h in range(H):
                op = pps.tile([BPC, HD], F32, tag="s", name="op")
                for u in range(CCH):
                    nc.tensor.matmul(
                        op,
                        lhsT=axT_sb[:, u, h * BPC : (h + 1) * BPC],
                        rhs=vw_sb[:, u, h * HD : (h + 1) * HD],
                        start=(u == 0),
                        stop=(u == CCH - 1),
                    )
                nc.scalar.copy(out=o_sb[:, h * HD : (h + 1) * HD], in_=op)
            nc.vector.tensor_add(out=o_sb, in0=o_sb, in1=vb_sb)

            otp = pps.tile([P, CCH, BPC], F32, tag="s", name="otp")
            for u in range(CCH):
                nc.tensor.transpose(
                    out=otp[:, u, :], in_=o_sb[:, ts(u, P)], identity=ident[:BPC, :BPC]
                )
            oT_sb = psm.tile([P, CCH, BPC], F32, name="oT_sb")
            nc.scalar.copy(out=oT_sb, in_=otp)

            xcls_sb = psm2.tile([BPC, C], F32, tag="sm", name="xcls_sb")
            for j in range(2):
                pp2 = pps.tile([BPC, 384], F32, tag="s", name="pp2")
                for u in range(CCH):
                    nc.tensor.matmul(
                        pp2,
                        lhsT=oT_sb[:, u, :],
                        rhs=pjw_sb[:, u, ts(j, 384)],
                        start=(u == 0),
                        stop=(u == CCH - 1),
                    )
                nc.scalar.copy(out=xcls_sb[:, ts(j, 384)], in_=pp2)
            nc.vector.tensor_add(out=xcls_sb, in0=xcls_sb, in1=pb_sb)
            nc.sync.dma_start(out=xcls_d, in_=xcls_sb)

    return nc


_CACHE = {}


def _get_nc():
    if "nc" not in _CACHE:
        nc = bacc.Bacc("TRN2", target_bir_lowering=False, debug=False,
                       num_devices=NCORES)
        build_kernel(nc)
        nc.compile()
        _CACHE["nc"] = nc
    return _CACHE["nc"]


def kernel(**inputs):
    from concourse.bass_utils import run_bass_kernel_spmd

    nc = _get_nc()
    x = np.ascontiguousarray(np.asarray(inputs["x"], dtype=np.float32))
    shared = {
        k: np.ascontiguousarray(np.asarray(inputs[k], dtype=np.float32))
        for k in ("q_w", "k_w", "v_w", "proj_w", "q_b", "v_b", "proj_b")
    }
    in_maps = []
    for c in range(NCORES):
        m = dict(shared)
        m["xs"] = np.ascontiguousarray(x[c * BPC : (c + 1) * BPC])
        in_maps.append(m)

    res = run_bass_kernel_spmd(nc, in_maps, core_ids=list(range(NCORES)))
    x_cls = np.concatenate([r["x_cls"] for r in res.results], axis=0)  # [B, C]
    attn = np.concatenate([r["attn"] for r in res.results], axis=0)    # [B, H, N]
    return (
        x_cls.reshape(B, 1, C).astype(np.float32),
        attn.reshape(B, H, 1, N).astype(np.float32),
    )
